# revision 1
# baseline (speedup 1.0000x reference)
"""Trainium2 Bass kernel for BiasedMHA (B=4, N=1024, C=1024, H=16, D=64).

Sharding: 8 cores = 4 batches x 2 head-halves (8 heads each).
Each core computes q/k/v projections for its head slice, biased softmax
attention, and its partial o_proj.  Host sums the two partials per batch
and adds the (bo + bv @ Wo.T) constant.

v4: all matmul operands fp16 (PE streams 2-byte moving operands at
1 cycle/column vs 2 for fp32/fp32r; accumulation stays fp32 in PSUM).
attn_bias is added by an identity-matmul accumulating into the score
PSUM (start=True), so the Vector engine is out of the attention inner
loop entirely (HW power management clamps the PE to half speed when
DVE/GpSimd duty is high, so elementwise offloads are net losers); exp
reads PSUM directly.  Projections run ct-outer over jt-pairs with
DMAs enqueued in consumption order, so the first matmul starts ~2us
in.  The all-zero q/k biases are skipped at build time (host checks).
o_proj runs kt-outer (shared stationary); attT is one tile per pair
to decouple o_proj's dependencies from the last pair's normalize.

Layouts (host-prepped, contraction-on-partitions):
  xT    [C, N]  fp16 : x[b].T
  wqT   [C, 512] fp16: Wq[rows,:].T * scale (softmax scale folded here)
  wkT/wvT [C, 512] fp16
  woT   [512, C] fp16: Wo[:, cols].T
  biasT [8, N, N] fp16: attn_bias[b, heads].transpose(0,2,1)  ([h, m, n])
  bqr/bkr [1, 512] fp16: bias rows, added via K=1 matmuls with a ones row
  ident [128, 128] fp16: identity (stationary operand of the bias-add mm)
  madd  [128, 8] fp32: additive mask (-1e30 where attn_mask==0), m-tiled

Attention per head: S^T[m, n] accumulates ident.T @ biasT (start=True)
then k^T(d,m).T @ q^T(d,n) (head pairs row-packed, K=64 at array rows
0-63/64-127).  Exp on ACT straight from PSUM (mask as per-partition
bias; no max-subtraction -- scores are O(+-7)).  P@V consumes expS^T
directly; v carries a ones column per head so PV row 64 is the softmax
denominator.  Normalize via reciprocal_approx_fast + ones-broadcast
matmul, multiply into attT[j, n] (fp16), then o_proj; partials returned
fp16 and summed on host in fp32.
"""

import sys

if "/opt/trn_rl_repo" not in sys.path:
    sys.path.insert(0, "/opt/trn_rl_repo")

from contextlib import ExitStack

import numpy as np

B, N, C, H = 4, 1024, 1024, 16
D = C // H            # 64
HL = H // 2           # 8 local heads per core
JL = HL * D           # 512 local head dims
NT = N // 128         # 8 seq tiles
CT = C // 128         # 8 contraction tiles
SCALE = D ** (-0.5)

_prog_cache = {}


def build_program(with_qk_bias=True):
    import concourse.tile as tile
    from concourse import bacc, mybir
    f32 = mybir.dt.float32
    f16 = mybir.dt.float16

    nc = bacc.Bacc("TRN2", target_bir_lowering=False, debug=False,
                   enable_asserts=False, num_devices=8)

    xT = nc.dram_tensor("xT", [C, N], f16, kind="ExternalInput").ap()
    wqT = nc.dram_tensor("wqT", [C, JL], f16, kind="ExternalInput").ap()
    wkT = nc.dram_tensor("wkT", [C, JL], f16, kind="ExternalInput").ap()
    wvT = nc.dram_tensor("wvT", [C, JL], f16, kind="ExternalInput").ap()
    woT = nc.dram_tensor("woT", [JL, C], f16, kind="ExternalInput").ap()
    bqr = nc.dram_tensor("bqr", [1, JL], f16, kind="ExternalInput").ap()
    bkr = nc.dram_tensor("bkr", [1, JL], f16, kind="ExternalInput").ap()
    ident = nc.dram_tensor("ident", [128, 128], f16, kind="ExternalInput").ap()
    biasT = nc.dram_tensor("biasT", [HL, N, N], f16, kind="ExternalInput").ap()
    madd = nc.dram_tensor("madd", [128, NT], f32, kind="ExternalInput").ap()
    outp = nc.dram_tensor("outp", [N, C], f16, kind="ExternalOutput").ap()

    Exp = mybir.ActivationFunctionType.Exp
    mult_op = mybir.AluOpType.mult
    add_op = mybir.AluOpType.add
    from concourse.tile_rust import add_dep_helper
    first_evac = [None]

    with tile.TileContext(nc) as tc, ExitStack() as ctx:
        # ---- pools ----
        resident = ctx.enter_context(tc.tile_pool(name="resident", bufs=1))
        biaspool = ctx.enter_context(tc.tile_pool(name="bias", bufs=12))
        exppool = ctx.enter_context(tc.tile_pool(name="exps", bufs=10))
        outpool = ctx.enter_context(tc.tile_pool(name="outs", bufs=3))
        smallpool = ctx.enter_context(tc.tile_pool(name="small", bufs=4))
        small2 = ctx.enter_context(tc.tile_pool(name="small2", bufs=6))
        ps_main = ctx.enter_context(
            tc.tile_pool(name="ps_main", bufs=2, space="PSUM"))
        ps_pv = ctx.enter_context(
            tc.tile_pool(name="ps_pv", bufs=2, space="PSUM"))

        # ---- resident tiles ----
        # chunked + consumption-ordered loads: DMA queues are global FIFOs,
        # so enqueue bytes in the order the projections will need them
        xts = resident.tile([128, CT, N], f16)          # xT tiled on c
        wq_sb = resident.tile([128, CT, JL], f16)       # wqT tiled on c
        wk_sb = resident.tile([128, CT, JL], f16)       # wkT tiled on c
        wv_sb = resident.tile([128, CT, JL], f16)       # wvT tiled on c
        wo_sb = resident.tile([128, 4, C], f16)         # woT tiled on j
        xview = xT.rearrange("(ct p) n -> p ct n", p=128)
        wqview = wqT.rearrange("(ct p) j -> p ct j", p=128)
        # DMAs arrive in the exact order the ct-outer projection loop
        # consumes them: per ct, the first jt-pair's wq columns then the
        # x chunk.  The projection's first matmul can start ~2us in.
        d0 = None
        for ct in range(CT):
            nc.sync.dma_start(wq_sb[:, ct, 0:256], wqview[:, ct, 0:256])
            last = nc.sync.dma_start(xts[:, ct, :], xview[:, ct, :])
            if d0 is None:
                d0 = last
        bulk = [
            nc.sync.dma_start(wq_sb[:, :, 256:512], wqview[:, :, 256:512]),
            nc.sync.dma_start(wk_sb[:],
                              wkT.rearrange("(ct p) j -> p ct j", p=128)),
            nc.sync.dma_start(wv_sb[:],
                              wvT.rearrange("(ct p) j -> p ct j", p=128)),
            nc.sync.dma_start(wo_sb[:],
                              woT.rearrange("(kt p) c -> p kt c", p=128)),
        ]
        for b in bulk:
            add_dep_helper(b.ins, d0.ins, reason="startup DMA priority")

        id_sb = resident.tile([128, 128], f16)
        nc.sync.dma_start(id_sb[:], ident)

        madd_sb = resident.tile([128, NT], f32)
        nc.sync.dma_start(madd_sb[:], madd)

        bq_sb = resident.tile([1, JL], f16)
        nc.sync.dma_start(bq_sb[:], bqr)
        bk_sb = resident.tile([1, JL], f16)
        nc.sync.dma_start(bk_sb[:], bkr)

        ones_f32 = resident.tile([128, 1], f32)
        nc.vector.memset(ones_f32[:], 1.0)
        ones_row = resident.tile([1, N], f16)
        nc.vector.tensor_copy(
            ones_row[:], ones_f32[0:1, 0:1].to_broadcast([1, N]))

        qT_sb = resident.tile([128, 4, N], f16)         # [j-tile, n]
        kT_sb = resident.tile([128, 4, N], f16)
        v_sb = resident.tile([128, NT, HL * (D + 1)], f16)  # [m-tile, h*65]
        # attT as one tile per pair so o_proj's early kt matmuls don't
        # transitively wait on the last pair's normalize
        attT_t = [resident.tile([128, N], f16, name=f"attT_{i}")
                  for i in range(4)]

        # ones columns of v (softmax denominator trick)
        for mt in range(NT):
            v4 = v_sb[:, mt, :].rearrange("p (h c) -> p h c", c=D + 1)
            nc.vector.tensor_copy(
                v4[:, :, D:D + 1],
                ones_f32[:, 0:1, None].to_broadcast([128, HL, 1]))

        # ---- PE warm-up: spin the HAM activity window while DMAs land ----
        warm_sb = resident.tile([128, 512], f16)
        nc.vector.memset(warm_sb[:], 0.0)
        warm_ps = ps_pv.tile([128, 512], f32, tag="pv", name="warm")
        for i in range(4):
            nc.tensor.matmul(warm_ps[:], warm_sb[:, 0:128], warm_sb[:],
                             start=True, stop=True)

        # ---- phase 1: projections ----
        # q/k transposed: out[j-tile, n] = sum_c wT[c, j] * xT[c, n] (+ bias)
        # ct-outer over jt-pairs so the first matmuls only need the first
        # w/x chunks (DMA-arrival order matches consumption order).
        for (wsb, brow, dest) in ((wq_sb, bq_sb, qT_sb), (wk_sb, bk_sb, kT_sb)):
            for jp in range(2):
                ps2 = [ps_main.tile([128, N], f32, tag="mm",
                                    name=f"proj_{jp}_{i}") for i in range(2)]
                for ct in range(CT):
                    for ji in range(2):
                        jt = jp * 2 + ji
                        w = wsb[:, ct, jt * 128:(jt + 1) * 128]
                        for nh in range(2):
                            nc.tensor.matmul(
                                ps2[ji][:, nh * 512:(nh + 1) * 512],
                                w[:],
                                xts[:, ct, nh * 512:(nh + 1) * 512],
                                start=(ct == 0),
                                stop=(not with_qk_bias and ct == CT - 1))
                for ji in range(2):
                    jt = jp * 2 + ji
                    # bias via K=1 matmul: ones over n, bias row over j
                    # (skipped when the host detects all-zero q/k biases)
                    if with_qk_bias:
                        for nh in range(2):
                            nc.tensor.matmul(
                                ps2[ji][:, nh * 512:(nh + 1) * 512],
                                brow[0:1, jt * 128:(jt + 1) * 128],
                                ones_row[0:1, nh * 512:(nh + 1) * 512],
                                start=False, stop=True)
                    ev = nc.vector.tensor_copy(dest[:, jt, :], ps2[ji][:])
                    if first_evac[0] is None:
                        first_evac[0] = ev

        # v normal layout: out[m-tile, j] = sum_c xT[c, m] * wvT[c, j]
        for mt in range(NT):
            ps = ps_main.tile([128, N], f32, tag="mm")
            psv = ps[:, 0:JL]
            for ct in range(CT):
                nc.tensor.matmul(
                    psv,
                    xts[:, ct, mt * 128:(mt + 1) * 128],
                    wv_sb[:, ct, :],
                    start=(ct == 0), stop=(ct == CT - 1))
            v4 = v_sb[:, mt, :].rearrange("p (h c) -> p h c", c=D + 1)
            nc.vector.tensor_copy(
                v4[:, :, 0:D],
                psv.rearrange("p (h c) -> p h c", c=D))

        # ---- phase 2: attention, one head pair at a time ----
        # Software-pipelined: each pair's first two m-tiles of bias/S/exp
        # are emitted before the previous pair's normalize, so the PE's
        # in-order stream has ready work while the DVE reciprocal chain
        # runs at the pair boundary.
        def emit_scores(hp, mt, s_tiles):
            hA, hB = 2 * hp, 2 * hp + 1
            bt = [None, None]
            for hi, h in enumerate((hA, hB)):
                b_ = biaspool.tile([128, N], f16, tag="bias",
                                   name=f"bias_{hp}_{mt}_{hi}")
                bdma = nc.gpsimd.dma_start(
                    b_[:], biasT[h, mt * 128:(mt + 1) * 128, :])
                if hp == 0 and first_evac[0] is not None:
                    add_dep_helper(bdma.ins, first_evac[0].ins,
                                   reason="bias prefetch behind startup loads")
                bt[hi] = b_
            sps = []
            for hi in range(2):
                sp = ps_main.tile([128, N], f32, tag="mm",
                                  name=f"s_{hp}_{mt}_{hi}")
                sps.append(sp)
            for hi in range(2):
                for nh in range(2):
                    sl = slice(nh * 512, (nh + 1) * 512)
                    nc.tensor.matmul(sps[hi][:, sl], id_sb[:], bt[hi][:, sl],
                                     start=True, stop=False)
            # S matmuls in alternating row groups (rows 0-63 / 64-127) so
            # each K=64 pair runs concurrently in the PE array
            for nh in range(2):
                sl = slice(nh * 512, (nh + 1) * 512)
                for hi in range(2):
                    base = hi * 64
                    nc.tensor.matmul(
                        sps[hi][:, sl],
                        kT_sb[base:base + 64, hp, mt * 128:(mt + 1) * 128],
                        qT_sb[base:base + 64, hp, sl],
                        start=False, stop=True)
            out = []
            for hi in range(2):
                et = exppool.tile([128, N], f16, tag="exp",
                                  name=f"exp_{hp}_{mt}_{hi}")
                nc.scalar.activation(et[:], sps[hi][:], Exp,
                                     bias=madd_sb[:, mt:mt + 1])
                out.append(et)
            return out

        def emit_pv(hp, mt, pv, ets):
            for hi, h in enumerate((2 * hp, 2 * hp + 1)):
                vx = v_sb[:, mt, h * 65:(h + 1) * 65]
                for nh in range(2):
                    nc.tensor.matmul(
                        pv[hi][0:65, nh * 512:(nh + 1) * 512],
                        vx,
                        ets[hi][:, nh * 512:(nh + 1) * 512],
                        start=(mt == 0), stop=(mt == NT - 1))

        def emit_normalize_a(hp, pv):
            # DVE-only prefix: drain pv PSUM (den + body copies) and build
            # the reciprocals; no PE instructions, so PE work emitted after
            # this overlaps the DVE chain
            den, recip32, recip16, body = {}, {}, {}, {}
            for hi in range(2):
                den[hi] = smallpool.tile([1, N], f32, tag="den",
                                         name=f"den_{hp}_{hi}")
                nc.vector.tensor_copy(den[hi][:], pv[hi][64:65, :])
            for hi in range(2):
                body[hi] = small2.tile([64, N], f16, tag="body",
                                       name=f"body_{hp}_{hi}")
                nc.vector.tensor_copy(body[hi][:], pv[hi][0:64, :])
            for hi in range(2):
                recip32[hi] = smallpool.tile([1, N], f32, tag="recip32",
                                             name=f"r32_{hp}_{hi}")
                nc.vector.reciprocal_approx_fast(
                    out=recip32[hi][:], in_=den[hi][:])
            for hi in range(2):
                recip16[hi] = small2.tile([1, N], f16, tag="recip16",
                                          name=f"r16_{hp}_{hi}")
                nc.vector.tensor_copy(recip16[hi][:], recip32[hi][:])
            return recip16, body

        def emit_normalize_b(hp, pv, recip16, body):
            # both broadcasts go through pv[1]'s rows 64:128 so pv[0] is
            # fully drained by its den+body copies and recycles ~2.5us
            # earlier for the next pair's first PV matmuls
            bc_sb = {}
            for hi in range(2):
                for nh in range(2):
                    nc.tensor.matmul(
                        pv[1][64:128, nh * 512:(nh + 1) * 512],
                        ones_row[0:1, 0:64],
                        recip16[hi][0:1, nh * 512:(nh + 1) * 512],
                        start=True, stop=True, tile_position=(0, 64))
                bc_sb[hi] = small2.tile([64, N], f16, tag="bcast",
                                        name=f"bc_{hp}_{hi}")
                nc.vector.tensor_copy(bc_sb[hi][:], pv[1][64:128, :])
            for hi in range(2):
                nc.vector.tensor_tensor(
                    attT_t[hp][hi * 64:hi * 64 + 64, :],
                    body[hi][:], bc_sb[hi][:], mult_op)

        def emit_normalize(hp, pv):
            recip16, body = emit_normalize_a(hp, pv)
            emit_normalize_b(hp, pv, recip16, body)

        # Flat lag-1 pipeline over all (pair, mt) steps: PV for step g-1 is
        # emitted alongside scores for step g, so the PE never head-of-line
        # waits on the exp of the current step.
        pv_by_pair = {}
        pending = []
        for g in range(4 * NT):
            hp, mt = divmod(g, NT)
            if mt == 0:
                pv_by_pair[hp] = [
                    ps_pv.tile([128, N], f32, tag="pv", name=f"pv_{hp}_{i}")
                    for i in range(2)]
            ets = emit_scores(hp, mt, None)
            pending.append((hp, mt, ets))
            if g >= 1:
                php, pmt, pets = pending.pop(0)
                emit_pv(php, pmt, pv_by_pair[php], pets)
                if pmt == NT - 1:
                    emit_normalize(php, pv_by_pair[php])
        php, pmt, pets = pending.pop(0)
        emit_pv(php, pmt, pv_by_pair[php], pets)
        emit_normalize(php, pv_by_pair[php])

        # ---- phase 3: o_proj partial ----
        # kt-outer so both ch matmuls share the same stationary attT slice
        for nt in range(NT):
            ps = ps_main.tile([128, N], f32, tag="mm")
            for kt in range(4):
                att = attT_t[kt][:, nt * 128:(nt + 1) * 128]
                for ch in range(2):
                    nc.tensor.matmul(
                        ps[:, ch * 512:(ch + 1) * 512],
                        att,
                        wo_sb[:, kt, ch * 512:(ch + 1) * 512],
                        start=(kt == 0), stop=(kt == 3))
            ot = outpool.tile([128, N], f16, tag="out")
            if nt == NT - 1:
                # split the last tile's evac+store so the DMA of the first
                # half overlaps the evacuation of the second (shorter tail)
                for h2 in range(2):
                    sl = slice(h2 * 512, (h2 + 1) * 512)
                    nc.vector.tensor_copy(ot[:, sl], ps[:, sl])
                    nc.sync.dma_start(outp[nt * 128:(nt + 1) * 128, sl],
                                      ot[:, sl])
            else:
                nc.vector.tensor_copy(ot[:], ps[:])
                nc.sync.dma_start(outp[nt * 128:(nt + 1) * 128, :], ot[:])

    nc.compile()
    return nc


def get_program(with_qk_bias=True):
    key = ("nc", with_qk_bias)
    if key not in _prog_cache:
        _prog_cache[key] = build_program(with_qk_bias=with_qk_bias)
    return _prog_cache[key]


def make_in_maps(x, attn_bias, attn_mask, Wq, bq, Wk, bk, Wv, bv, Wo, bo):
    """Host-side shard + layout prep.  Returns (in_maps, const) where
    const[c_out] = bo + bv @ Wo.T must be added to the gathered output."""
    x = np.asarray(x, np.float32)
    attn_bias = np.asarray(attn_bias, np.float32)
    attn_mask = np.asarray(attn_mask)
    Wq = np.asarray(Wq, np.float32)
    Wk = np.asarray(Wk, np.float32)
    Wv = np.asarray(Wv, np.float32)
    Wo = np.asarray(Wo, np.float32)
    bq = np.asarray(bq, np.float32)
    bk = np.asarray(bk, np.float32)
    bv = np.asarray(bv, np.float32)
    bo = np.asarray(bo, np.float32)

    const = bo + bv @ Wo.T
    ident = np.eye(128, dtype=np.float16)

    xTs = [np.ascontiguousarray(x[b].T).astype(np.float16) for b in range(B)]
    madds = []
    for b in range(B):
        ma = np.where(attn_mask[b] == 0, np.float32(-1e30), np.float32(0.0))
        madds.append(np.ascontiguousarray(ma.reshape(NT, 128).T))

    in_maps = []
    for core in range(8):
        b, half = divmod(core, 2)
        rows = slice(half * JL, (half + 1) * JL)
        wqT = np.ascontiguousarray(
            (Wq[rows, :] * np.float32(SCALE)).T).astype(np.float16)
        wkT = np.ascontiguousarray(Wk[rows, :].T).astype(np.float16)
        wvT = np.ascontiguousarray(Wv[rows, :].T).astype(np.float16)
        woT = np.ascontiguousarray(Wo[:, rows].T).astype(np.float16)
        bqr = (bq[rows] * np.float32(SCALE)).reshape(1, JL).astype(np.float16)
        bkr = bk[rows].reshape(1, JL).astype(np.float16)
        bT = np.ascontiguousarray(
            attn_bias[b, half * HL:(half + 1) * HL].transpose(0, 2, 1)
        ).astype(np.float16)
        in_maps.append({
            "xT": xTs[b], "wqT": wqT, "wkT": wkT, "wvT": wvT, "woT": woT,
            "bqr": bqr, "bkr": bkr, "ident": ident, "biasT": bT,
            "madd": madds[b],
        })
    return in_maps, const


def gather(results, const):
    out = np.empty((B, N, C), np.float32)
    for b in range(B):
        out[b] = results[2 * b]["outp"].astype(np.float32) \
            + results[2 * b + 1]["outp"].astype(np.float32) \
            + const[None, :]
    return out


def kernel(**inputs):
    from concourse.bass_utils import run_bass_kernel_spmd
    wb = bool(np.any(np.asarray(inputs["bq"]))
              or np.any(np.asarray(inputs["bk"])))
    nc = get_program(with_qk_bias=wb)
    in_maps, const = make_in_maps(**inputs)
    res = run_bass_kernel_spmd(nc, in_maps, core_ids=list(range(8)))
    return gather(res.results, const)



# revision 4
# speedup vs baseline: 1.0226x; 1.0226x over previous
"""Trainium2 Bass kernel for BiasedMHA (B=4, N=1024, C=1024, H=16, D=64).

Sharding: 8 cores = 4 batches x 2 head-halves (8 heads each).
Each core computes q/k/v projections for its head slice, biased softmax
attention, and its partial o_proj.  Host sums the two partials per batch
and adds the (bo + bv @ Wo.T) constant.

v5 (cost-model-driven rewrite of v4):
- attn_bias folded as exp(S+B) = exp(S) * exp(B): host precomputes
  exp(B) fp16; a DVE fp16 multiply (2x mode) replaces the identity
  matmul bias-add, removing ~32us of PE column-streams.
- wide denominator: the PV stationary carries 64 ones-columns, so PSUM
  rows 64:127 hold the softmax denominator replicated across 64
  partitions.  Normalize is then reciprocal_approx_fast straight off
  PSUM + one PSUM-source tensor_tensor multiply into attT -- no [1,N]
  partition-starved chain, no broadcast matmuls, no copies.
- exp stays on ACT (bias = per-partition mask add); the two heads of a
  step share one ets tile so the expB multiply runs per-head right
  after its exp (finer pipeline).
- q/k/v evacuations alternate DVE / ACT-Copy (ACT is idle in phase A).
- v ones-columns via one GpSimd memset of the whole v buffer (GpSimd
  is otherwise only a DMA trigger engine).
- o_proj evacs alternate ACT/DVE; last tile split for DMA overlap.

Layouts (host-prepped, contraction-on-partitions):
  xT    [C, N]  fp16 : x[b].T
  wqT   [C, 512] fp16: Wq[rows,:].T * scale (softmax scale folded)
  wkT/wvT [C, 512] fp16
  woT   [512, C] fp16: Wo[:, cols].T
  expbT [4, N, 2048] fp16: exp(attn_bias)[b, pair-heads].T packed
        [hp, m, headA-n | headB-n]
  bqr/bkr [1, 512] fp16: bias rows, added via K=1 matmuls (skipped
        when host detects all-zero q/k biases)
  madd  [128, NT] fp32: additive mask (-1e30 where attn_mask==0)
"""

import sys

if "/opt/trn_rl_repo" not in sys.path:
    sys.path.insert(0, "/opt/trn_rl_repo")

from contextlib import ExitStack

import numpy as np

B, N, C, H = 4, 1024, 1024, 16
D = C // H            # 64
HL = H // 2           # 8 local heads per core
JL = HL * D           # 512 local head dims
NT = N // 128         # 8 seq tiles
CT = C // 128         # 8 contraction tiles
SCALE = D ** (-0.5)

_prog_cache = {}


def build_program(with_qk_bias=True):
    import concourse.tile as tile
    from concourse import bacc, mybir
    f32 = mybir.dt.float32
    f16 = mybir.dt.float16

    nc = bacc.Bacc("TRN2", target_bir_lowering=False, debug=False,
                   enable_asserts=False, num_devices=8)

    xT = nc.dram_tensor("xT", [C, N], f16, kind="ExternalInput").ap()
    wqT = nc.dram_tensor("wqT", [C, JL], f16, kind="ExternalInput").ap()
    wkT = nc.dram_tensor("wkT", [C, JL], f16, kind="ExternalInput").ap()
    wvT = nc.dram_tensor("wvT", [C, JL], f16, kind="ExternalInput").ap()
    woT = nc.dram_tensor("woT", [JL, C], f16, kind="ExternalInput").ap()
    bqr = nc.dram_tensor("bqr", [1, JL], f16, kind="ExternalInput").ap()
    bkr = nc.dram_tensor("bkr", [1, JL], f16, kind="ExternalInput").ap()
    expbT = nc.dram_tensor("expbT", [4, N, 2 * N], f16,
                           kind="ExternalInput").ap()
    madd = nc.dram_tensor("madd", [128, NT], f32, kind="ExternalInput").ap()
    outp = nc.dram_tensor("outp", [N, C], f16, kind="ExternalOutput").ap()

    Exp = mybir.ActivationFunctionType.Exp
    Copy = mybir.ActivationFunctionType.Copy
    mult_op = mybir.AluOpType.mult
    from concourse.tile_rust import add_dep_helper
    first_evac = [None]

    with tile.TileContext(nc) as tc, ExitStack() as ctx:
        # ---- pools ----
        resident = ctx.enter_context(tc.tile_pool(name="resident", bufs=1))
        biaspool = ctx.enter_context(tc.tile_pool(name="bias", bufs=5))
        exppool = ctx.enter_context(tc.tile_pool(name="exps", bufs=4))
        outpool = ctx.enter_context(tc.tile_pool(name="outs", bufs=3))
        smallpool = ctx.enter_context(tc.tile_pool(name="small", bufs=4))
        ps_main = ctx.enter_context(
            tc.tile_pool(name="ps_main", bufs=2, space="PSUM"))
        ps_pv = ctx.enter_context(
            tc.tile_pool(name="ps_pv", bufs=2, space="PSUM"))

        # ---- resident tiles ----
        # chunked + consumption-ordered loads: DMA queues are global FIFOs,
        # so enqueue bytes in the order the projections will need them
        xts = resident.tile([128, CT, N], f16)          # xT tiled on c
        wq_sb = resident.tile([128, CT, JL], f16)       # wqT tiled on c
        wk_sb = resident.tile([128, CT, JL], f16)       # wkT tiled on c
        wv_sb = resident.tile([128, CT, JL], f16)       # wvT tiled on c
        wo_sb = resident.tile([128, 4, C], f16)         # woT tiled on j
        xview = xT.rearrange("(ct p) n -> p ct n", p=128)
        wqview = wqT.rearrange("(ct p) j -> p ct j", p=128)
        d0 = None
        for ct in range(CT):
            nc.sync.dma_start(wq_sb[:, ct, 0:256], wqview[:, ct, 0:256])
            last = nc.sync.dma_start(xts[:, ct, :], xview[:, ct, :])
            if d0 is None:
                d0 = last
        bulk = [
            nc.sync.dma_start(wq_sb[:, :, 256:512], wqview[:, :, 256:512]),
            nc.sync.dma_start(wk_sb[:],
                              wkT.rearrange("(ct p) j -> p ct j", p=128)),
            nc.sync.dma_start(wv_sb[:],
                              wvT.rearrange("(ct p) j -> p ct j", p=128)),
            nc.sync.dma_start(wo_sb[:],
                              woT.rearrange("(kt p) c -> p kt c", p=128)),
        ]
        for b in bulk:
            add_dep_helper(b.ins, d0.ins, reason="startup DMA priority")

        madd_sb = resident.tile([128, NT], f32)
        nc.sync.dma_start(madd_sb[:], madd)

        bq_sb = resident.tile([1, JL], f16)
        nc.sync.dma_start(bq_sb[:], bqr)
        bk_sb = resident.tile([1, JL], f16)
        nc.sync.dma_start(bk_sb[:], bkr)

        qT_sb = resident.tile([128, 4, N], f16)         # [j-tile, n]
        kT_sb = resident.tile([128, 4, N], f16)
        # v with 64 ones-columns per head: [m-tile, h*(64 V + 64 ones)]
        v_sb = resident.tile([128, NT, HL * 2 * D], f16)
        # attT as one tile per pair so o_proj's early kt matmuls don't
        # transitively wait on the last pair's normalize
        attT_t = [resident.tile([128, N], f16, name=f"attT_{i}")
                  for i in range(4)]

        ones_f32 = resident.tile([128, 1], f32)
        nc.vector.memset(ones_f32[:], 1.0)
        ones_row = None
        if with_qk_bias:
            ones_row = resident.tile([1, N], f16)
            nc.vector.tensor_copy(
                ones_row[:], ones_f32[0:1, 0:1].to_broadcast([1, N]))

        # ones columns of v (wide softmax-denominator trick)
        for mt in range(NT):
            v4o = v_sb[:, mt, :].rearrange("p (h c) -> p h c", c=2 * D)
            nc.vector.tensor_copy(
                v4o[:, :, D:2 * D],
                ones_f32[:, 0:1, None].to_broadcast([128, HL, D]))

        # ---- PE warm-up: spin the p-state ramp while DMAs land ----
        warm_sb = resident.tile([128, 512], f16)
        nc.vector.memset(warm_sb[:], 0.0)
        warm_ps = ps_pv.tile([128, 512], f32, tag="pv", name="warm")
        for i in range(4):
            nc.tensor.matmul(warm_ps[:], warm_sb[:, 0:128], warm_sb[:],
                             start=True, stop=True)

        # ---- phase A: projections ----
        # q/k transposed: out[j-tile, n] = sum_c wT[c, j] * xT[c, n] (+ bias)
        # ct-outer over jt-pairs so the first matmuls only need the first
        # w/x chunks (DMA-arrival order matches consumption order).
        evac_ctr = [0]

        def evac(dst, src):
            # alternate DVE / ACT so neither engine gates the projections
            ev = evac_ctr[0]
            evac_ctr[0] += 1
            if ev % 2 == 0:
                return nc.vector.tensor_copy(dst, src)
            return nc.scalar.activation(dst, src, Copy)

        for (wsb, brow, dest) in ((wq_sb, bq_sb, qT_sb), (wk_sb, bk_sb, kT_sb)):
            for jp in range(2):
                ps2 = [ps_main.tile([128, N], f32, tag="mm",
                                    name=f"proj_{jp}_{i}") for i in range(2)]
                for ct in range(CT):
                    for ji in range(2):
                        jt = jp * 2 + ji
                        w = wsb[:, ct, jt * 128:(jt + 1) * 128]
                        for nh in range(2):
                            nc.tensor.matmul(
                                ps2[ji][:, nh * 512:(nh + 1) * 512],
                                w[:],
                                xts[:, ct, nh * 512:(nh + 1) * 512],
                                start=(ct == 0),
                                stop=(not with_qk_bias and ct == CT - 1))
                for ji in range(2):
                    jt = jp * 2 + ji
                    if with_qk_bias:
                        for nh in range(2):
                            nc.tensor.matmul(
                                ps2[ji][:, nh * 512:(nh + 1) * 512],
                                brow[0:1, jt * 128:(jt + 1) * 128],
                                ones_row[0:1, nh * 512:(nh + 1) * 512],
                                start=False, stop=True)
                    ev = evac(dest[:, jt, :], ps2[ji][:])
                    if first_evac[0] is None:
                        first_evac[0] = ev

        # v normal layout: out[m-tile, j] = sum_c xT[c, m] * wvT[c, j]
        for mt in range(NT):
            ps = ps_main.tile([128, N], f32, tag="mm")
            psv = ps[:, 0:JL]
            for ct in range(CT):
                nc.tensor.matmul(
                    psv,
                    xts[:, ct, mt * 128:(mt + 1) * 128],
                    wv_sb[:, ct, :],
                    start=(ct == 0), stop=(ct == CT - 1))
            v4 = v_sb[:, mt, :].rearrange("p (h c) -> p h c", c=2 * D)
            evac(v4[:, :, 0:D], psv.rearrange("p (h c) -> p h c", c=D))

        # ---- phase B: attention, lag-1 pipelined over (pair, mt) steps ----
        def emit_scores(hp, mt):
            eb = biaspool.tile([128, 2 * N], f16, tag="bias",
                               name=f"expb_{hp}_{mt}")
            bdma = nc.gpsimd.dma_start(
                eb[:], expbT[hp, mt * 128:(mt + 1) * 128, :])
            if hp == 0 and first_evac[0] is not None:
                add_dep_helper(bdma.ins, first_evac[0].ins,
                               reason="bias prefetch behind startup loads")
            sps = [ps_main.tile([128, N], f32, tag="mm",
                                name=f"s_{hp}_{mt}_{hi}") for hi in range(2)]
            for nh in range(2):
                sl = slice(nh * 512, (nh + 1) * 512)
                for hi in range(2):
                    base = hi * 64
                    nc.tensor.matmul(
                        sps[hi][:, sl],
                        kT_sb[base:base + 64, hp, mt * 128:(mt + 1) * 128],
                        qT_sb[base:base + 64, hp, sl],
                        start=True, stop=True)
            et = exppool.tile([128, 2 * N], f16, tag="exp",
                              name=f"exp_{hp}_{mt}")
            for hi in range(2):
                sl = slice(hi * N, (hi + 1) * N)
                nc.scalar.activation(et[:, sl], sps[hi][:], Exp,
                                     bias=madd_sb[:, mt:mt + 1])
                # fold exp(bias) on DVE (fp16 2x mode), in place
                nc.vector.tensor_tensor(et[:, sl], et[:, sl], eb[:, sl],
                                        mult_op)
            return et

        def emit_pv(hp, mt, pv, et):
            for hi in range(2):
                h = 2 * hp + hi
                vx = v_sb[:, mt, h * 128:(h + 1) * 128]
                for nh in range(2):
                    sl = slice(hi * N + nh * 512, hi * N + (nh + 1) * 512)
                    nc.tensor.matmul(
                        pv[hi][:, nh * 512:(nh + 1) * 512],
                        vx,
                        et[:, sl],
                        start=(mt == 0), stop=(mt == NT - 1))

        def emit_normalize(hp, pv):
            # rows 64:127 of pv hold the denominator replicated
            dens, bodys, rs = [], [], []
            for hi in range(2):
                d_ = smallpool.tile([64, N], f32, tag="den",
                                    name=f"den_{hp}_{hi}")
                nc.vector.tensor_copy(d_[:], pv[hi][64:128, :])
                dens.append(d_)
                b_ = smallpool.tile([64, N], f16, tag="body",
                                    name=f"body_{hp}_{hi}")
                nc.vector.tensor_copy(b_[:], pv[hi][0:64, :])
                bodys.append(b_)
            for hi in range(2):
                r = smallpool.tile([64, N], f32, tag="recip",
                                   name=f"recip_{hp}_{hi}")
                nc.vector.reciprocal_approx_fast(out=r[:], in_=dens[hi][:])
                rs.append(r)
            for hi in range(2):
                nc.vector.tensor_tensor(
                    attT_t[hp][hi * 64:(hi + 1) * 64, :],
                    bodys[hi][:], rs[hi][:], mult_op)

        pv_by_pair = {}
        pending = []
        for g in range(4 * NT):
            hp, mt = divmod(g, NT)
            if mt == 0:
                pv_by_pair[hp] = [
                    ps_pv.tile([128, N], f32, tag="pv", name=f"pv_{hp}_{i}")
                    for i in range(2)]
            et = emit_scores(hp, mt)
            pending.append((hp, mt, et))
            if g >= 1:
                php, pmt, pet = pending.pop(0)
                emit_pv(php, pmt, pv_by_pair[php], pet)
                if pmt == NT - 1:
                    emit_normalize(php, pv_by_pair[php])
        php, pmt, pet = pending.pop(0)
        emit_pv(php, pmt, pv_by_pair[php], pet)
        emit_normalize(php, pv_by_pair[php])

        # ---- phase C: o_proj partial ----
        # kt-outer so both ch matmuls share the same stationary attT slice
        for nt in range(NT):
            ps = ps_main.tile([128, N], f32, tag="mm")
            for kt in range(4):
                att = attT_t[kt][:, nt * 128:(nt + 1) * 128]
                for ch in range(2):
                    nc.tensor.matmul(
                        ps[:, ch * 512:(ch + 1) * 512],
                        att,
                        wo_sb[:, kt, ch * 512:(ch + 1) * 512],
                        start=(kt == 0), stop=(kt == 3))
            ot = outpool.tile([128, N], f16, tag="out")
            if nt == NT - 1:
                # split the last tile's evac+store so the DMA of the first
                # half overlaps the evacuation of the second (shorter tail)
                for h2 in range(2):
                    sl = slice(h2 * 512, (h2 + 1) * 512)
                    ev = (nc.scalar.activation(ot[:, sl], ps[:, sl], Copy)
                          if h2 == 0 else
                          nc.vector.tensor_copy(ot[:, sl], ps[:, sl]))
                    nc.sync.dma_start(outp[nt * 128:(nt + 1) * 128, sl],
                                      ot[:, sl])
            else:
                if nt % 2 == 0:
                    nc.scalar.activation(ot[:], ps[:], Copy)
                else:
                    nc.vector.tensor_copy(ot[:], ps[:])
                nc.sync.dma_start(outp[nt * 128:(nt + 1) * 128, :], ot[:])

    nc.compile()
    return nc


def get_program(with_qk_bias=True):
    key = ("nc", with_qk_bias)
    if key not in _prog_cache:
        _prog_cache[key] = build_program(with_qk_bias=with_qk_bias)
    return _prog_cache[key]


def make_in_maps(x, attn_bias, attn_mask, Wq, bq, Wk, bk, Wv, bv, Wo, bo):
    """Host-side shard + layout prep.  Returns (in_maps, const) where
    const[c_out] = bo + bv @ Wo.T must be added to the gathered output."""
    x = np.asarray(x, np.float32)
    attn_bias = np.asarray(attn_bias, np.float32)
    attn_mask = np.asarray(attn_mask)
    Wq = np.asarray(Wq, np.float32)
    Wk = np.asarray(Wk, np.float32)
    Wv = np.asarray(Wv, np.float32)
    Wo = np.asarray(Wo, np.float32)
    bq = np.asarray(bq, np.float32)
    bk = np.asarray(bk, np.float32)
    bv = np.asarray(bv, np.float32)
    bo = np.asarray(bo, np.float32)

    const = bo + bv @ Wo.T

    xTs = [np.ascontiguousarray(x[b].T).astype(np.float16) for b in range(B)]
    madds = []
    for b in range(B):
        ma = np.where(attn_mask[b] == 0, np.float32(-1e30), np.float32(0.0))
        madds.append(np.ascontiguousarray(ma.reshape(NT, 128).T))

    # exp of bias, transposed to [h, m(key), n(query)], packed per pair:
    # expbT[hp, m, 0:N] = head 2hp, [hp, m, N:2N] = head 2hp+1
    expb = np.exp(attn_bias).astype(np.float16)  # [B, H, n, m]

    in_maps = []
    for core in range(8):
        b, half = divmod(core, 2)
        rows = slice(half * JL, (half + 1) * JL)
        wqT = np.ascontiguousarray(
            (Wq[rows, :] * np.float32(SCALE)).T).astype(np.float16)
        wkT = np.ascontiguousarray(Wk[rows, :].T).astype(np.float16)
        wvT = np.ascontiguousarray(Wv[rows, :].T).astype(np.float16)
        woT = np.ascontiguousarray(Wo[:, rows].T).astype(np.float16)
        bqr = (bq[rows] * np.float32(SCALE)).reshape(1, JL).astype(np.float16)
        bkr = bk[rows].reshape(1, JL).astype(np.float16)
        # [HL, n, m] -> [HL, m, n] -> pairs packed on last axis
        eT = expb[b, half * HL:(half + 1) * HL].transpose(0, 2, 1)
        ebT = np.empty((4, N, 2 * N), np.float16)
        for hp in range(4):
            ebT[hp, :, 0:N] = eT[2 * hp]
            ebT[hp, :, N:2 * N] = eT[2 * hp + 1]
        in_maps.append({
            "xT": xTs[b], "wqT": wqT, "wkT": wkT, "wvT": wvT, "woT": woT,
            "bqr": bqr, "bkr": bkr, "expbT": ebT,
            "madd": madds[b],
        })
    return in_maps, const


def gather(results, const):
    out = np.empty((B, N, C), np.float32)
    for b in range(B):
        out[b] = results[2 * b]["outp"].astype(np.float32) \
            + results[2 * b + 1]["outp"].astype(np.float32) \
            + const[None, :]
    return out


def kernel(**inputs):
    from concourse.bass_utils import run_bass_kernel_spmd
    wb = bool(np.any(np.asarray(inputs["bq"]))
              or np.any(np.asarray(inputs["bk"])))
    nc = get_program(with_qk_bias=wb)
    in_maps, const = make_in_maps(**inputs)
    res = run_bass_kernel_spmd(nc, in_maps, core_ids=list(range(8)))
    return gather(res.results, const)


# revision 6
# speedup vs baseline: 1.0382x; 1.0152x over previous
"""Trainium2 Bass kernel for BiasedMHA (B=4, N=1024, C=1024, H=16, D=64).

Sharding: 8 cores = 4 batches x 2 head-halves (8 heads each).
Each core computes q/k/v projections for its head slice, biased softmax
attention, and its partial o_proj.  Host sums the two partials per batch
and adds the (bo + bv @ Wo.T) constant.

v5 (cost-model-driven rewrite of v4):
- attn_bias folded as exp(S+B) = exp(S) * exp(B): host precomputes
  exp(B) fp16; a DVE fp16 multiply (2x mode) replaces the identity
  matmul bias-add, removing ~32us of PE column-streams.
- wide denominator: the PV stationary carries 64 ones-columns, so PSUM
  rows 64:127 hold the softmax denominator replicated across 64
  partitions.  Normalize is then reciprocal_approx_fast straight off
  PSUM + one PSUM-source tensor_tensor multiply into attT -- no [1,N]
  partition-starved chain, no broadcast matmuls, no copies.
- exp stays on ACT (bias = per-partition mask add); the two heads of a
  step share one ets tile so the expB multiply runs per-head right
  after its exp (finer pipeline).
- q/k/v evacuations alternate DVE / ACT-Copy (ACT is idle in phase A).
- v ones-columns via one GpSimd memset of the whole v buffer (GpSimd
  is otherwise only a DMA trigger engine).
- o_proj evacs alternate ACT/DVE; last tile split for DMA overlap.

Layouts (host-prepped, contraction-on-partitions):
  xT    [C, N]  fp16 : x[b].T
  wqT   [C, 512] fp16: Wq[rows,:].T * scale (softmax scale folded)
  wkT/wvT [C, 512] fp16
  woT   [512, C] fp16: Wo[:, cols].T
  expbT [4, N, 2048] fp16: exp(attn_bias)[b, pair-heads].T packed
        [hp, m, headA-n | headB-n]
  bqr/bkr [1, 512] fp16: bias rows, added via K=1 matmuls (skipped
        when host detects all-zero q/k biases)
  madd  [128, NT] fp32: additive mask (-1e30 where attn_mask==0)
"""

import sys

if "/opt/trn_rl_repo" not in sys.path:
    sys.path.insert(0, "/opt/trn_rl_repo")

from contextlib import ExitStack

import numpy as np

B, N, C, H = 4, 1024, 1024, 16
D = C // H            # 64
HL = H // 2           # 8 local heads per core
JL = HL * D           # 512 local head dims
NT = N // 128         # 8 seq tiles
CT = C // 128         # 8 contraction tiles
SCALE = D ** (-0.5)

_prog_cache = {}


def build_program(with_qk_bias=True):
    import concourse.tile as tile
    from concourse import bacc, mybir
    f32 = mybir.dt.float32
    f16 = mybir.dt.float16

    nc = bacc.Bacc("TRN2", target_bir_lowering=False, debug=False,
                   enable_asserts=False, num_devices=8)

    xT = nc.dram_tensor("xT", [C, N], f16, kind="ExternalInput").ap()
    wqT = nc.dram_tensor("wqT", [C, JL], f16, kind="ExternalInput").ap()
    wkT = nc.dram_tensor("wkT", [C, JL], f16, kind="ExternalInput").ap()
    wvT = nc.dram_tensor("wvT", [C, JL], f16, kind="ExternalInput").ap()
    woT = nc.dram_tensor("woT", [JL, C], f16, kind="ExternalInput").ap()
    bqr = nc.dram_tensor("bqr", [1, JL], f16, kind="ExternalInput").ap()
    bkr = nc.dram_tensor("bkr", [1, JL], f16, kind="ExternalInput").ap()
    expbT = nc.dram_tensor("expbT", [4, N, 2 * N], f16,
                           kind="ExternalInput").ap()
    madd = nc.dram_tensor("madd", [128, NT], f32, kind="ExternalInput").ap()
    outp = nc.dram_tensor("outp", [N, C], f16, kind="ExternalOutput").ap()

    Exp = mybir.ActivationFunctionType.Exp
    Copy = mybir.ActivationFunctionType.Copy
    mult_op = mybir.AluOpType.mult
    from concourse.tile_rust import add_dep_helper
    first_evac = [None]

    with tile.TileContext(nc) as tc, ExitStack() as ctx:
        # ---- pools ----
        resident = ctx.enter_context(tc.tile_pool(name="resident", bufs=1))
        biaspool = ctx.enter_context(tc.tile_pool(name="bias", bufs=5))
        exppool = ctx.enter_context(tc.tile_pool(name="exps", bufs=4))
        outpool = ctx.enter_context(tc.tile_pool(name="outs", bufs=3))
        smallpool = ctx.enter_context(tc.tile_pool(name="small", bufs=4))
        ps_main = ctx.enter_context(
            tc.tile_pool(name="ps_main", bufs=2, space="PSUM"))
        ps_pv = ctx.enter_context(
            tc.tile_pool(name="ps_pv", bufs=2, space="PSUM"))

        # ---- resident tiles ----
        # chunked + consumption-ordered loads: DMA queues are global FIFOs,
        # so enqueue bytes in the order the projections will need them
        xts = resident.tile([128, CT, N], f16)          # xT tiled on c
        wq_sb = resident.tile([128, CT, JL], f16)       # wqT tiled on c
        wk_sb = resident.tile([128, CT, JL], f16)       # wkT tiled on c
        wv_sb = resident.tile([128, CT, JL], f16)       # wvT tiled on c
        wo_sb = resident.tile([128, 4, C], f16)         # woT tiled on j
        xview = xT.rearrange("(ct p) n -> p ct n", p=128)
        wqview = wqT.rearrange("(ct p) j -> p ct j", p=128)
        d0 = None
        for ct in range(CT):
            nc.sync.dma_start(wq_sb[:, ct, 0:256], wqview[:, ct, 0:256])
            last = nc.sync.dma_start(xts[:, ct, :], xview[:, ct, :])
            if d0 is None:
                d0 = last
        bulk = [
            nc.sync.dma_start(wq_sb[:, :, 256:512], wqview[:, :, 256:512]),
            nc.sync.dma_start(wk_sb[:],
                              wkT.rearrange("(ct p) j -> p ct j", p=128)),
            nc.sync.dma_start(wv_sb[:],
                              wvT.rearrange("(ct p) j -> p ct j", p=128)),
            nc.sync.dma_start(wo_sb[:],
                              woT.rearrange("(kt p) c -> p kt c", p=128)),
        ]
        for b in bulk:
            add_dep_helper(b.ins, d0.ins, reason="startup DMA priority")

        madd_sb = resident.tile([128, NT], f32)
        nc.sync.dma_start(madd_sb[:], madd)

        bq_sb = resident.tile([1, JL], f16)
        nc.sync.dma_start(bq_sb[:], bqr)
        bk_sb = resident.tile([1, JL], f16)
        nc.sync.dma_start(bk_sb[:], bkr)

        qT_sb = resident.tile([128, 4, N], f16)         # [j-tile, n]
        kT_sb = resident.tile([128, 4, N], f16)
        # v with 64 ones-columns per head: [m-tile, h*(64 V + 64 ones)]
        v_sb = resident.tile([128, NT, HL * 2 * D], f16)
        # attT as one tile per pair so o_proj's early kt matmuls don't
        # transitively wait on the last pair's normalize
        attT_t = [resident.tile([128, N], f16, name=f"attT_{i}")
                  for i in range(4)]

        ones_f32 = resident.tile([128, 1], f32)
        nc.vector.memset(ones_f32[:], 1.0)
        ones_row = None
        if with_qk_bias:
            ones_row = resident.tile([1, N], f16)
            nc.vector.tensor_copy(
                ones_row[:], ones_f32[0:1, 0:1].to_broadcast([1, N]))

        # ones columns of v (wide softmax-denominator trick)
        for mt in range(NT):
            v4o = v_sb[:, mt, :].rearrange("p (h c) -> p h c", c=2 * D)
            nc.vector.tensor_copy(
                v4o[:, :, D:2 * D],
                ones_f32[:, 0:1, None].to_broadcast([128, HL, D]))

        # ---- PE warm-up: spin the p-state ramp while DMAs land ----
        warm_sb = resident.tile([128, 512], f16)
        nc.vector.memset(warm_sb[:], 0.0)
        warm_ps = ps_pv.tile([128, 512], f32, tag="pv", name="warm")
        for i in range(4):
            nc.tensor.matmul(warm_ps[:], warm_sb[:, 0:128], warm_sb[:],
                             start=True, stop=True)

        # ---- phase A: projections ----
        # q/k transposed: out[j-tile, n] = sum_c wT[c, j] * xT[c, n] (+ bias)
        # ct-outer over jt-pairs so the first matmuls only need the first
        # w/x chunks (DMA-arrival order matches consumption order).
        evac_ctr = [0]

        def evac(dst, src):
            # alternate DVE / ACT so neither engine gates the projections
            ev = evac_ctr[0]
            evac_ctr[0] += 1
            if ev % 2 == 0:
                return nc.vector.tensor_copy(dst, src)
            return nc.scalar.activation(dst, src, Copy)

        def emit_qk_group(wsb, brow, dest, jp):
            ps2 = [ps_main.tile([128, N], f32, tag="mm",
                                name=f"proj_{jp}_{i}") for i in range(2)]
            for ct in range(CT):
                for ji in range(2):
                    jt = jp * 2 + ji
                    w = wsb[:, ct, jt * 128:(jt + 1) * 128]
                    for nh in range(2):
                        nc.tensor.matmul(
                            ps2[ji][:, nh * 512:(nh + 1) * 512],
                            w[:],
                            xts[:, ct, nh * 512:(nh + 1) * 512],
                            start=(ct == 0),
                            stop=(not with_qk_bias and ct == CT - 1))
            for ji in range(2):
                jt = jp * 2 + ji
                if with_qk_bias:
                    for nh in range(2):
                        nc.tensor.matmul(
                            ps2[ji][:, nh * 512:(nh + 1) * 512],
                            brow[0:1, jt * 128:(jt + 1) * 128],
                            ones_row[0:1, nh * 512:(nh + 1) * 512],
                            start=False, stop=True)
                ev = evac(dest[:, jt, :], ps2[ji][:])
                if first_evac[0] is None:
                    first_evac[0] = ev

        def emit_v_group(mt):
            # v normal layout: out[m-tile, j] = sum_c xT[c, m] * wvT[c, j]
            # psum borrowed from the (idle until attention) ps_pv pool so v
            # matmuls fill the PE while q/k psum groups drain
            ps = ps_pv.tile([128, N], f32, tag="pv", name=f"vproj_{mt}")
            psv = ps[:, 0:JL]
            for ct in range(CT):
                nc.tensor.matmul(
                    psv,
                    xts[:, ct, mt * 128:(mt + 1) * 128],
                    wv_sb[:, ct, :],
                    start=(ct == 0), stop=(ct == CT - 1))
            v4 = v_sb[:, mt, :].rearrange("p (h c) -> p h c", c=2 * D)
            evac(v4[:, :, 0:D], psv.rearrange("p (h c) -> p h c", c=D))

        emit_qk_group(wq_sb, bq_sb, qT_sb, 0)
        emit_v_group(0)
        emit_v_group(1)
        emit_qk_group(wq_sb, bq_sb, qT_sb, 1)
        emit_v_group(2)
        emit_v_group(3)
        emit_qk_group(wk_sb, bk_sb, kT_sb, 0)
        emit_v_group(4)
        emit_v_group(5)
        emit_qk_group(wk_sb, bk_sb, kT_sb, 1)
        emit_v_group(6)
        emit_v_group(7)

        # ---- phase B: attention, lag-1 pipelined over (pair, mt) steps ----
        def emit_scores(hp, mt):
            eb = biaspool.tile([128, 2 * N], f16, tag="bias",
                               name=f"expb_{hp}_{mt}")
            bdma = nc.gpsimd.dma_start(
                eb[:], expbT[hp, mt * 128:(mt + 1) * 128, :])
            if hp == 0 and first_evac[0] is not None:
                add_dep_helper(bdma.ins, first_evac[0].ins,
                               reason="bias prefetch behind startup loads")
            sps = [ps_main.tile([128, N], f32, tag="mm",
                                name=f"s_{hp}_{mt}_{hi}") for hi in range(2)]
            # keep-warm filler: the PE p-state ramp demotes to half speed
            # after any idle; phase B is ACT-bound, so burn the slack on
            # dummy matmuls into the about-to-be-overwritten S psum (the
            # real S matmul start=True resets it)
            for hi in range(2):
                nc.tensor.matmul(
                    sps[hi][:, 0:512], warm_sb[:, 0:128], warm_sb[:],
                    start=True, stop=True, skip_group_check=True)
            for nh in range(2):
                sl = slice(nh * 512, (nh + 1) * 512)
                for hi in range(2):
                    base = hi * 64
                    nc.tensor.matmul(
                        sps[hi][:, sl],
                        kT_sb[base:base + 64, hp, mt * 128:(mt + 1) * 128],
                        qT_sb[base:base + 64, hp, sl],
                        start=True, stop=True, skip_group_check=True)
            et = exppool.tile([128, 2 * N], f16, tag="exp",
                              name=f"exp_{hp}_{mt}")
            for hi in range(2):
                sl = slice(hi * N, (hi + 1) * N)
                nc.scalar.activation(et[:, sl], sps[hi][:], Exp,
                                     bias=madd_sb[:, mt:mt + 1])
                # fold exp(bias) on DVE (fp16 2x mode), in place
                nc.vector.tensor_tensor(et[:, sl], et[:, sl], eb[:, sl],
                                        mult_op)
            return et

        def emit_pv(hp, mt, pv, et):
            for hi in range(2):
                h = 2 * hp + hi
                vx = v_sb[:, mt, h * 128:(h + 1) * 128]
                for nh in range(2):
                    sl = slice(hi * N + nh * 512, hi * N + (nh + 1) * 512)
                    nc.tensor.matmul(
                        pv[hi][:, nh * 512:(nh + 1) * 512],
                        vx,
                        et[:, sl],
                        start=(mt == 0), stop=(mt == NT - 1))

        def emit_normalize(hp, pv):
            # rows 64:127 of pv hold the denominator replicated
            dens, bodys, rs = [], [], []
            for hi in range(2):
                d_ = smallpool.tile([64, N], f32, tag="den",
                                    name=f"den_{hp}_{hi}")
                nc.vector.tensor_copy(d_[:], pv[hi][64:128, :])
                dens.append(d_)
                b_ = smallpool.tile([64, N], f16, tag="body",
                                    name=f"body_{hp}_{hi}")
                nc.vector.tensor_copy(b_[:], pv[hi][0:64, :])
                bodys.append(b_)
            for hi in range(2):
                r = smallpool.tile([64, N], f32, tag="recip",
                                   name=f"recip_{hp}_{hi}")
                nc.vector.reciprocal_approx_fast(out=r[:], in_=dens[hi][:])
                rs.append(r)
            for hi in range(2):
                nc.vector.tensor_tensor(
                    attT_t[hp][hi * 64:(hi + 1) * 64, :],
                    bodys[hi][:], rs[hi][:], mult_op)

        pv_by_pair = {}
        pending = []
        for g in range(4 * NT):
            hp, mt = divmod(g, NT)
            if mt == 0:
                pv_by_pair[hp] = [
                    ps_pv.tile([128, N], f32, tag="pv", name=f"pv_{hp}_{i}")
                    for i in range(2)]
            et = emit_scores(hp, mt)
            pending.append((hp, mt, et))
            if g >= 1:
                php, pmt, pet = pending.pop(0)
                emit_pv(php, pmt, pv_by_pair[php], pet)
                if pmt == NT - 1:
                    emit_normalize(php, pv_by_pair[php])
        php, pmt, pet = pending.pop(0)
        emit_pv(php, pmt, pv_by_pair[php], pet)
        emit_normalize(php, pv_by_pair[php])

        # ---- phase C: o_proj partial ----
        # kt-outer so both ch matmuls share the same stationary attT slice
        for nt in range(NT):
            ps = ps_main.tile([128, N], f32, tag="mm")
            for kt in range(4):
                att = attT_t[kt][:, nt * 128:(nt + 1) * 128]
                for ch in range(2):
                    nc.tensor.matmul(
                        ps[:, ch * 512:(ch + 1) * 512],
                        att,
                        wo_sb[:, kt, ch * 512:(ch + 1) * 512],
                        start=(kt == 0), stop=(kt == 3))
            ot = outpool.tile([128, N], f16, tag="out")
            if nt == NT - 1:
                # split the last tile's evac+store so the DMA of the first
                # half overlaps the evacuation of the second (shorter tail)
                for h2 in range(2):
                    sl = slice(h2 * 512, (h2 + 1) * 512)
                    ev = (nc.scalar.activation(ot[:, sl], ps[:, sl], Copy)
                          if h2 == 0 else
                          nc.vector.tensor_copy(ot[:, sl], ps[:, sl]))
                    nc.sync.dma_start(outp[nt * 128:(nt + 1) * 128, sl],
                                      ot[:, sl])
            else:
                if nt % 2 == 0:
                    nc.scalar.activation(ot[:], ps[:], Copy)
                else:
                    nc.vector.tensor_copy(ot[:], ps[:])
                nc.sync.dma_start(outp[nt * 128:(nt + 1) * 128, :], ot[:])

    nc.compile()
    return nc


def get_program(with_qk_bias=True):
    key = ("nc", with_qk_bias)
    if key not in _prog_cache:
        _prog_cache[key] = build_program(with_qk_bias=with_qk_bias)
    return _prog_cache[key]


def make_in_maps(x, attn_bias, attn_mask, Wq, bq, Wk, bk, Wv, bv, Wo, bo):
    """Host-side shard + layout prep.  Returns (in_maps, const) where
    const[c_out] = bo + bv @ Wo.T must be added to the gathered output."""
    x = np.asarray(x, np.float32)
    attn_bias = np.asarray(attn_bias, np.float32)
    attn_mask = np.asarray(attn_mask)
    Wq = np.asarray(Wq, np.float32)
    Wk = np.asarray(Wk, np.float32)
    Wv = np.asarray(Wv, np.float32)
    Wo = np.asarray(Wo, np.float32)
    bq = np.asarray(bq, np.float32)
    bk = np.asarray(bk, np.float32)
    bv = np.asarray(bv, np.float32)
    bo = np.asarray(bo, np.float32)

    const = bo + bv @ Wo.T

    xTs = [np.ascontiguousarray(x[b].T).astype(np.float16) for b in range(B)]
    madds = []
    for b in range(B):
        ma = np.where(attn_mask[b] == 0, np.float32(-1e30), np.float32(0.0))
        madds.append(np.ascontiguousarray(ma.reshape(NT, 128).T))

    # exp of bias, transposed to [h, m(key), n(query)], packed per pair:
    # expbT[hp, m, 0:N] = head 2hp, [hp, m, N:2N] = head 2hp+1
    expb = np.exp(attn_bias).astype(np.float16)  # [B, H, n, m]

    in_maps = []
    for core in range(8):
        b, half = divmod(core, 2)
        rows = slice(half * JL, (half + 1) * JL)
        wqT = np.ascontiguousarray(
            (Wq[rows, :] * np.float32(SCALE)).T).astype(np.float16)
        wkT = np.ascontiguousarray(Wk[rows, :].T).astype(np.float16)
        wvT = np.ascontiguousarray(Wv[rows, :].T).astype(np.float16)
        woT = np.ascontiguousarray(Wo[:, rows].T).astype(np.float16)
        bqr = (bq[rows] * np.float32(SCALE)).reshape(1, JL).astype(np.float16)
        bkr = bk[rows].reshape(1, JL).astype(np.float16)
        # [HL, n, m] -> [HL, m, n] -> pairs packed on last axis
        eT = expb[b, half * HL:(half + 1) * HL].transpose(0, 2, 1)
        ebT = np.empty((4, N, 2 * N), np.float16)
        for hp in range(4):
            ebT[hp, :, 0:N] = eT[2 * hp]
            ebT[hp, :, N:2 * N] = eT[2 * hp + 1]
        in_maps.append({
            "xT": xTs[b], "wqT": wqT, "wkT": wkT, "wvT": wvT, "woT": woT,
            "bqr": bqr, "bkr": bkr, "expbT": ebT,
            "madd": madds[b],
        })
    return in_maps, const


def gather(results, const):
    out = np.empty((B, N, C), np.float32)
    for b in range(B):
        out[b] = results[2 * b]["outp"].astype(np.float32) \
            + results[2 * b + 1]["outp"].astype(np.float32) \
            + const[None, :]
    return out


def kernel(**inputs):
    from concourse.bass_utils import run_bass_kernel_spmd
    wb = bool(np.any(np.asarray(inputs["bq"]))
              or np.any(np.asarray(inputs["bk"])))
    nc = get_program(with_qk_bias=wb)
    in_maps, const = make_in_maps(**inputs)
    res = run_bass_kernel_spmd(nc, in_maps, core_ids=list(range(8)))
    return gather(res.results, const)


# revision 7
# speedup vs baseline: 1.0632x; 1.0241x over previous
"""Trainium2 Bass kernel for BiasedMHA (B=4, N=1024, C=1024, H=16, D=64).

Sharding: 8 cores = 4 batches x 2 head-halves (8 heads each).
Each core computes q/k/v projections for its head slice, biased softmax
attention, and its partial o_proj.  Host sums the two partials per batch
and adds the (bo + bv @ Wo.T) constant.

v7: PE-density-first design.  The PE p-state ramp demotes to ~2x cycle
time after ANY idle and needs >3us of continuous execution to recover,
so every phase is arranged to keep the PE the strict bottleneck with
zero stalls:
- attn_bias is added by an identity-matmul accumulating into the score
  PSUM (start=True) -- useful PE filler that also keeps exp reading
  S+B straight from PSUM (no elementwise engines in the attention
  inner loop).
- wide denominator: the PV stationary carries 64 ones-columns, so PSUM
  rows 64:127 hold the softmax denominator replicated.  Normalize is
  den-copy/body-copy/recip/mult on DVE only -- no PE instruction at
  the pair boundary to stall on.
- phase A (projections) alternates q/k psum groups and v psum groups
  across both PSUM pools (4-deep pipelining, no evac-WAR stalls);
  evacuations alternate DVE / ACT-Copy.
- phase C (o_proj) alternates groups across both PSUM pools so the PE
  stream never micro-stalls (which would pin it at mid p-state).

Layouts (host-prepped, contraction-on-partitions):
  xT    [C, N]  fp16 : x[b].T
  wqT   [C, 512] fp16: Wq[rows,:].T * scale (softmax scale folded)
  wkT/wvT [C, 512] fp16
  woT   [512, C] fp16: Wo[:, cols].T
  biasT [4, N, 2048] fp16: attn_bias[b, pair].T packed
        [hp, m, headA-n | headB-n]
  bqr/bkr [1, 512] fp16: bias rows via K=1 matmuls (skipped when the
        host detects all-zero q/k biases)
  ident [128, 128] fp16: identity (stationary of the bias-add matmul)
  madd  [128, NT] fp32: additive mask (-1e30 where attn_mask==0)
"""

import sys

if "/opt/trn_rl_repo" not in sys.path:
    sys.path.insert(0, "/opt/trn_rl_repo")

from contextlib import ExitStack

import numpy as np

B, N, C, H = 4, 1024, 1024, 16
D = C // H            # 64
HL = H // 2           # 8 local heads per core
JL = HL * D           # 512 local head dims
NT = N // 128         # 8 seq tiles
CT = C // 128         # 8 contraction tiles
SCALE = D ** (-0.5)

_prog_cache = {}


def build_program(with_qk_bias=True):
    import concourse.tile as tile
    from concourse import bacc, mybir
    f32 = mybir.dt.float32
    f16 = mybir.dt.float16

    nc = bacc.Bacc("TRN2", target_bir_lowering=False, debug=False,
                   enable_asserts=False, num_devices=8)

    xT = nc.dram_tensor("xT", [C, N], f16, kind="ExternalInput").ap()
    wqT = nc.dram_tensor("wqT", [C, JL], f16, kind="ExternalInput").ap()
    wkT = nc.dram_tensor("wkT", [C, JL], f16, kind="ExternalInput").ap()
    wvT = nc.dram_tensor("wvT", [C, JL], f16, kind="ExternalInput").ap()
    woT = nc.dram_tensor("woT", [JL, C], f16, kind="ExternalInput").ap()
    bqr = nc.dram_tensor("bqr", [1, JL], f16, kind="ExternalInput").ap()
    bkr = nc.dram_tensor("bkr", [1, JL], f16, kind="ExternalInput").ap()
    ident = nc.dram_tensor("ident", [128, 128], f16, kind="ExternalInput").ap()
    biasT = nc.dram_tensor("biasT", [4, N, 2 * N], f16,
                           kind="ExternalInput").ap()
    madd = nc.dram_tensor("madd", [128, NT], f32, kind="ExternalInput").ap()
    outp = nc.dram_tensor("outp", [N, C], f16, kind="ExternalOutput").ap()

    Exp = mybir.ActivationFunctionType.Exp
    Copy = mybir.ActivationFunctionType.Copy
    mult_op = mybir.AluOpType.mult
    from concourse.tile_rust import add_dep_helper
    first_evac = [None]

    with tile.TileContext(nc) as tc, ExitStack() as ctx:
        # ---- pools ----
        resident = ctx.enter_context(tc.tile_pool(name="resident", bufs=1))
        biaspool = ctx.enter_context(tc.tile_pool(name="bias", bufs=6))
        exppool = ctx.enter_context(tc.tile_pool(name="exps", bufs=5))
        outpool = ctx.enter_context(tc.tile_pool(name="outs", bufs=3))
        smallpool = ctx.enter_context(tc.tile_pool(name="small", bufs=6))
        ps_main = ctx.enter_context(
            tc.tile_pool(name="ps_main", bufs=2, space="PSUM"))
        ps_pv = ctx.enter_context(
            tc.tile_pool(name="ps_pv", bufs=2, space="PSUM"))

        # ---- resident tiles ----
        # chunked + consumption-ordered loads: DMA queues are global FIFOs,
        # so enqueue bytes in the order the projections will need them
        xts = resident.tile([128, CT, N], f16)          # xT tiled on c
        wq_sb = resident.tile([128, CT, JL], f16)       # wqT tiled on c
        wk_sb = resident.tile([128, CT, JL], f16)       # wkT tiled on c
        wv_sb = resident.tile([128, CT, JL], f16)       # wvT tiled on c
        wo_sb = resident.tile([128, 4, C], f16)         # woT tiled on j
        xview = xT.rearrange("(ct p) n -> p ct n", p=128)
        wqview = wqT.rearrange("(ct p) j -> p ct j", p=128)
        d0 = None
        for ct in range(CT):
            nc.sync.dma_start(wq_sb[:, ct, 0:256], wqview[:, ct, 0:256])
            last = nc.sync.dma_start(xts[:, ct, :], xview[:, ct, :])
            if d0 is None:
                d0 = last
        bulk = [
            nc.sync.dma_start(wq_sb[:, :, 256:512], wqview[:, :, 256:512]),
            nc.sync.dma_start(wk_sb[:],
                              wkT.rearrange("(ct p) j -> p ct j", p=128)),
            nc.sync.dma_start(wv_sb[:],
                              wvT.rearrange("(ct p) j -> p ct j", p=128)),
            nc.sync.dma_start(wo_sb[:],
                              woT.rearrange("(kt p) c -> p kt c", p=128)),
        ]
        for b in bulk:
            add_dep_helper(b.ins, d0.ins, reason="startup DMA priority")

        id_sb = resident.tile([128, 128], f16)
        nc.sync.dma_start(id_sb[:], ident)

        madd_sb = resident.tile([128, NT], f32)
        nc.sync.dma_start(madd_sb[:], madd)

        bq_sb = resident.tile([1, JL], f16)
        nc.sync.dma_start(bq_sb[:], bqr)
        bk_sb = resident.tile([1, JL], f16)
        nc.sync.dma_start(bk_sb[:], bkr)

        qT_sb = resident.tile([128, 4, N], f16)         # [j-tile, n]
        kT_sb = resident.tile([128, 4, N], f16)
        # v with 64 ones-columns per head: [m-tile, h*(64 V + 64 ones)]
        v_sb = resident.tile([128, NT, HL * 2 * D], f16)
        # attT as one tile per pair so o_proj's early kt matmuls don't
        # transitively wait on the last pair's normalize
        attT_t = [resident.tile([128, N], f16, name=f"attT_{i}")
                  for i in range(4)]

        ones_f32 = resident.tile([128, 1], f32)
        nc.vector.memset(ones_f32[:], 1.0)
        ones_row = None
        if with_qk_bias:
            ones_row = resident.tile([1, N], f16)
            nc.vector.tensor_copy(
                ones_row[:], ones_f32[0:1, 0:1].to_broadcast([1, N]))

        # ones columns of v (wide softmax-denominator trick)
        for mt in range(NT):
            v4o = v_sb[:, mt, :].rearrange("p (h c) -> p h c", c=2 * D)
            nc.vector.tensor_copy(
                v4o[:, :, D:2 * D],
                ones_f32[:, 0:1, None].to_broadcast([128, HL, D]))

        # ---- PE warm-up while the first DMAs land ----
        warm_sb = resident.tile([128, 512], f16)
        nc.vector.memset(warm_sb[:], 0.0)
        warm_ps = ps_pv.tile([128, 512], f32, tag="pv", name="warm")
        for i in range(4):
            nc.tensor.matmul(warm_ps[:], warm_sb[:, 0:128], warm_sb[:],
                             start=True, stop=True)

        # ---- phase A: projections, 4-deep across both PSUM pools ----
        evac_ctr = [0]

        def evac(dst, src):
            # alternate DVE / ACT so neither engine gates the projections
            ev = evac_ctr[0]
            evac_ctr[0] += 1
            if ev % 2 == 0:
                return nc.vector.tensor_copy(dst, src)
            return nc.scalar.activation(dst, src, Copy)

        pool_ctr = [0]

        def next_pool():
            pool_ctr[0] += 1
            return ps_main if pool_ctr[0] % 2 else ps_pv

        def emit_qk_group(wsb, brow, dest, jp):
            pool = next_pool()
            ps2 = [pool.tile([128, N], f32, tag="mm" if pool is ps_main
                             else "pv", name=f"proj_{jp}_{i}")
                   for i in range(2)]
            for ct in range(CT):
                for ji in range(2):
                    jt = jp * 2 + ji
                    w = wsb[:, ct, jt * 128:(jt + 1) * 128]
                    for nh in range(2):
                        nc.tensor.matmul(
                            ps2[ji][:, nh * 512:(nh + 1) * 512],
                            w[:],
                            xts[:, ct, nh * 512:(nh + 1) * 512],
                            start=(ct == 0),
                            stop=(not with_qk_bias and ct == CT - 1))
            for ji in range(2):
                jt = jp * 2 + ji
                if with_qk_bias:
                    for nh in range(2):
                        nc.tensor.matmul(
                            ps2[ji][:, nh * 512:(nh + 1) * 512],
                            brow[0:1, jt * 128:(jt + 1) * 128],
                            ones_row[0:1, nh * 512:(nh + 1) * 512],
                            start=False, stop=True)
                ev = evac(dest[:, jt, :], ps2[ji][:])
                if first_evac[0] is None:
                    first_evac[0] = ev

        def emit_v_group(mt):
            # v normal layout: out[m-tile, j] = sum_c xT[c, m] * wvT[c, j]
            pool = next_pool()
            ps = pool.tile([128, N], f32, tag="mm" if pool is ps_main
                           else "pv", name=f"vproj_{mt}")
            psv = ps[:, 0:JL]
            for ct in range(CT):
                nc.tensor.matmul(
                    psv,
                    xts[:, ct, mt * 128:(mt + 1) * 128],
                    wv_sb[:, ct, :],
                    start=(ct == 0), stop=(ct == CT - 1))
            v4 = v_sb[:, mt, :].rearrange("p (h c) -> p h c", c=2 * D)
            evac(v4[:, :, 0:D], psv.rearrange("p (h c) -> p h c", c=D))

        # DMA arrival order is wq+x, wk, wv -- consume in that order
        emit_qk_group(wq_sb, bq_sb, qT_sb, 0)
        emit_qk_group(wq_sb, bq_sb, qT_sb, 1)
        emit_qk_group(wk_sb, bk_sb, kT_sb, 0)
        emit_qk_group(wk_sb, bk_sb, kT_sb, 1)
        for mt in range(NT):
            emit_v_group(mt)

        # ---- phase B: attention, lag-1 pipelined over (pair, mt) steps ----
        def emit_scores(hp, mt):
            eb = biaspool.tile([128, 2 * N], f16, tag="bias",
                               name=f"bias_{hp}_{mt}")
            bdma = nc.gpsimd.dma_start(
                eb[:], biasT[hp, mt * 128:(mt + 1) * 128, :])
            if hp == 0 and first_evac[0] is not None:
                add_dep_helper(bdma.ins, first_evac[0].ins,
                               reason="bias prefetch behind startup loads")
            sps = [ps_main.tile([128, N], f32, tag="mm",
                                name=f"s_{hp}_{mt}_{hi}") for hi in range(2)]
            # bias lands in PSUM via identity matmul (start=True); the PE
            # is the bottleneck engine on purpose -- it must never idle
            for hi in range(2):
                for nh in range(2):
                    sl = slice(nh * 512, (nh + 1) * 512)
                    nc.tensor.matmul(
                        sps[hi][:, sl], id_sb[:],
                        eb[:, hi * N + nh * 512:hi * N + (nh + 1) * 512],
                        start=True, stop=False)
            for nh in range(2):
                sl = slice(nh * 512, (nh + 1) * 512)
                for hi in range(2):
                    base = hi * 64
                    nc.tensor.matmul(
                        sps[hi][:, sl],
                        kT_sb[base:base + 64, hp, mt * 128:(mt + 1) * 128],
                        qT_sb[base:base + 64, hp, sl],
                        start=False, stop=True)
            et = exppool.tile([128, 2 * N], f16, tag="exp",
                              name=f"exp_{hp}_{mt}")
            for hi in range(2):
                nc.scalar.activation(et[:, hi * N:(hi + 1) * N], sps[hi][:],
                                     Exp, bias=madd_sb[:, mt:mt + 1])
            return et

        def emit_pv(hp, mt, pv, et):
            for hi in range(2):
                h = 2 * hp + hi
                vx = v_sb[:, mt, h * 128:(h + 1) * 128]
                for nh in range(2):
                    sl = slice(hi * N + nh * 512, hi * N + (nh + 1) * 512)
                    nc.tensor.matmul(
                        pv[hi][:, nh * 512:(nh + 1) * 512],
                        vx,
                        et[:, sl],
                        start=(mt == 0), stop=(mt == NT - 1))

        def emit_normalize(hp, pv):
            # rows 64:127 of pv hold the denominator replicated; DVE-only
            # chain so the PE stream has nothing to stall on here
            dens, bodys, rs = [], [], []
            for hi in range(2):
                d_ = smallpool.tile([64, N], f32, tag="den",
                                    name=f"den_{hp}_{hi}")
                nc.vector.tensor_copy(d_[:], pv[hi][64:128, :])
                dens.append(d_)
                b_ = smallpool.tile([64, N], f16, tag="body",
                                    name=f"body_{hp}_{hi}")
                nc.vector.tensor_copy(b_[:], pv[hi][0:64, :])
                bodys.append(b_)
            for hi in range(2):
                r = smallpool.tile([64, N], f32, tag="recip",
                                   name=f"recip_{hp}_{hi}")
                nc.vector.reciprocal_approx_fast(out=r[:], in_=dens[hi][:])
                rs.append(r)
            for hi in range(2):
                nc.vector.tensor_tensor(
                    attT_t[hp][hi * 64:(hi + 1) * 64, :],
                    bodys[hi][:], rs[hi][:], mult_op)

        pv_by_pair = {}
        pending = []
        for g in range(4 * NT):
            hp, mt = divmod(g, NT)
            if mt == 0:
                pv_by_pair[hp] = [
                    ps_pv.tile([128, N], f32, tag="pv", name=f"pv_{hp}_{i}")
                    for i in range(2)]
            et = emit_scores(hp, mt)
            pending.append((hp, mt, et))
            if g >= 1:
                php, pmt, pet = pending.pop(0)
                emit_pv(php, pmt, pv_by_pair[php], pet)
                if pmt == NT - 1:
                    emit_normalize(php, pv_by_pair[php])
        php, pmt, pet = pending.pop(0)
        emit_pv(php, pmt, pv_by_pair[php], pet)
        emit_normalize(php, pv_by_pair[php])

        # ---- phase C: o_proj partial ----
        # kt-outer so both ch matmuls share the same stationary attT slice;
        # groups alternate across both PSUM pools so the in-order PE stream
        # never waits on an evacuation (micro-stalls would pin mid p-state)
        for nt in range(NT):
            pool = next_pool()
            ps = pool.tile([128, N], f32,
                           tag="mm" if pool is ps_main else "pv")
            for kt in range(4):
                att = attT_t[kt][:, nt * 128:(nt + 1) * 128]
                for ch in range(2):
                    nc.tensor.matmul(
                        ps[:, ch * 512:(ch + 1) * 512],
                        att,
                        wo_sb[:, kt, ch * 512:(ch + 1) * 512],
                        start=(kt == 0), stop=(kt == 3))
            ot = outpool.tile([128, N], f16, tag="out")
            if nt == NT - 1:
                # split the last tile's evac+store so the DMA of the first
                # half overlaps the evacuation of the second (shorter tail)
                for h2 in range(2):
                    sl = slice(h2 * 512, (h2 + 1) * 512)
                    ev = (nc.scalar.activation(ot[:, sl], ps[:, sl], Copy)
                          if h2 == 0 else
                          nc.vector.tensor_copy(ot[:, sl], ps[:, sl]))
                    nc.sync.dma_start(outp[nt * 128:(nt + 1) * 128, sl],
                                      ot[:, sl])
            else:
                if nt % 2 == 0:
                    nc.scalar.activation(ot[:], ps[:], Copy)
                else:
                    nc.vector.tensor_copy(ot[:], ps[:])
                nc.sync.dma_start(outp[nt * 128:(nt + 1) * 128, :], ot[:])

    nc.compile()
    return nc


def get_program(with_qk_bias=True):
    key = ("nc", with_qk_bias)
    if key not in _prog_cache:
        _prog_cache[key] = build_program(with_qk_bias=with_qk_bias)
    return _prog_cache[key]


def make_in_maps(x, attn_bias, attn_mask, Wq, bq, Wk, bk, Wv, bv, Wo, bo):
    """Host-side shard + layout prep.  Returns (in_maps, const) where
    const[c_out] = bo + bv @ Wo.T must be added to the gathered output."""
    x = np.asarray(x, np.float32)
    attn_bias = np.asarray(attn_bias, np.float32)
    attn_mask = np.asarray(attn_mask)
    Wq = np.asarray(Wq, np.float32)
    Wk = np.asarray(Wk, np.float32)
    Wv = np.asarray(Wv, np.float32)
    Wo = np.asarray(Wo, np.float32)
    bq = np.asarray(bq, np.float32)
    bk = np.asarray(bk, np.float32)
    bv = np.asarray(bv, np.float32)
    bo = np.asarray(bo, np.float32)

    const = bo + bv @ Wo.T
    ident = np.eye(128, dtype=np.float16)

    xTs = [np.ascontiguousarray(x[b].T).astype(np.float16) for b in range(B)]
    madds = []
    for b in range(B):
        ma = np.where(attn_mask[b] == 0, np.float32(-1e30), np.float32(0.0))
        madds.append(np.ascontiguousarray(ma.reshape(NT, 128).T))

    bias16 = attn_bias.astype(np.float16)  # [B, H, n, m]

    in_maps = []
    for core in range(8):
        b, half = divmod(core, 2)
        rows = slice(half * JL, (half + 1) * JL)
        wqT = np.ascontiguousarray(
            (Wq[rows, :] * np.float32(SCALE)).T).astype(np.float16)
        wkT = np.ascontiguousarray(Wk[rows, :].T).astype(np.float16)
        wvT = np.ascontiguousarray(Wv[rows, :].T).astype(np.float16)
        woT = np.ascontiguousarray(Wo[:, rows].T).astype(np.float16)
        bqr = (bq[rows] * np.float32(SCALE)).reshape(1, JL).astype(np.float16)
        bkr = bk[rows].reshape(1, JL).astype(np.float16)
        # [HL, n, m] -> [HL, m, n] -> pairs packed on last axis
        eT = bias16[b, half * HL:(half + 1) * HL].transpose(0, 2, 1)
        ebT = np.empty((4, N, 2 * N), np.float16)
        for hp in range(4):
            ebT[hp, :, 0:N] = eT[2 * hp]
            ebT[hp, :, N:2 * N] = eT[2 * hp + 1]
        in_maps.append({
            "xT": xTs[b], "wqT": wqT, "wkT": wkT, "wvT": wvT, "woT": woT,
            "bqr": bqr, "bkr": bkr, "ident": ident, "biasT": ebT,
            "madd": madds[b],
        })
    return in_maps, const


def gather(results, const):
    out = np.empty((B, N, C), np.float32)
    for b in range(B):
        out[b] = results[2 * b]["outp"].astype(np.float32) \
            + results[2 * b + 1]["outp"].astype(np.float32) \
            + const[None, :]
    return out


def kernel(**inputs):
    from concourse.bass_utils import run_bass_kernel_spmd
    wb = bool(np.any(np.asarray(inputs["bq"]))
              or np.any(np.asarray(inputs["bk"])))
    nc = get_program(with_qk_bias=wb)
    in_maps, const = make_in_maps(**inputs)
    res = run_bass_kernel_spmd(nc, in_maps, core_ids=list(range(8)))
    return gather(res.results, const)
